# revision 12
# baseline (speedup 1.0000x reference)
"""Trainium2 Bass kernel for nn_ComplexNN (3-layer MLP, blended tanh act).

  h1 = blend_act(x @ W1 + b1);  blend_act(z) = z>0 ? 0.9z+0.1tanh(z) : 0.5tanh(z)
  h2 = relu(h1 @ W2 + b2)
  out = h2 @ W3 + b3

Data-parallel over 8 NeuronCores: each core takes 4096 rows of x, weights
replicated. Fully fused on-chip; matmuls in bf16 with fp32 PSUM accumulate.

Layout: activations are kept feature-on-partitions (h1^T, h2^T) so each
matmul's contraction dim lands on partitions with no intermediate
transposes. x is cast fp32->bf16 via SWDGE DMA (DRAM->DRAM, whole-chunk)
then DMA-xbar-transposed (DRAM->SBUF). out^T [10, 4096] goes to DRAM and
the host transposes during the unshard/gather step.

Queue plan (no head-of-line blocking): gpsimd/SWDGE carries ONLY the x
casts (pure prefetch FIFO), sync HWDGE carries ONLY the xbar transposes,
scalar HWDGE carries weights (front) then output stores.

Pipeline: uniform 512-col chunks (256 head/tail). mm2's k-accumulation is
interleaved into the mm1 i-loop (lag 3) so the tensor queue never drains
at chunk tails. mm3 packs its tiny M=10 output into two PE column groups
(k-pairs run concurrently at col offsets 0/32), reduced on DVE.
A few warm-up matmuls on a memset tile run during ingest to lift the HAM
clock gate (PE starts at 1.2 GHz, un-throttles after ~3.4us of activity).

blend_act decomposition (t = tanh(z)):
  blend(z) = 0.9*relu(z) + 0.1*t + 0.4*min(t, 0)
ACT: t = Tanh(psum + b1);  a = Relu(0.9*psum + 0.9*b1)
DVE: m = (t min 0)*0.4 ;  u = 0.1*t + a (STT);  h1 = u + m
"""

import sys

sys.path.insert(0, "/opt/trn_rl_repo")

import ml_dtypes
import numpy as np

import concourse.bass as bass
import concourse.mybir as mybir
import concourse.tile as tile
from concourse import bacc
from concourse.bass_utils import run_bass_kernel_spmd

N_CORES = 8
B, D, H, H2, C = 32768, 512, 1024, 512, 10
BL = B // N_CORES  # rows per core = 4096
CHUNKS = [256, 256, 512, 512, 512, 512, 512, 512, 256, 256]
assert sum(CHUNKS) == BL
KD = D // 128      # 4  k-tiles for mm1
KH = H // 128      # 8  k-tiles for mm2 / h-tiles of h1
KH2 = H2 // 128    # 4  k-tiles for mm3 / h2-tiles of h2
NBMAX = max(CHUNKS)
PK1_BYTES = KH * KD * 256 + 8 * KH          # w1 bf16 | b1c f32 | b1s f32
PK2_BYTES = KH2 * KH * 256 + 4 * KH2 + 2 * KH2 * C + 4  # w2 | b2c | w3 | b3
WARM_MM = 6        # warm-up matmuls to release the HAM clock gate
LAG = 3            # mm2(k) issues after mm1(k+LAG)
MM3_TILED = True   # mm3 via 2 PE column groups

F32 = mybir.dt.float32
BF16 = mybir.dt.bfloat16
AF = mybir.ActivationFunctionType
ALU = mybir.AluOpType


def _body(ctx, tc, outs, ins):
    nc = tc.nc
    x, pk1, pk2 = ins
    (outT,) = outs

    wpool = ctx.enter_context(tc.tile_pool(name="weights", bufs=1))
    xpool = ctx.enter_context(tc.tile_pool(name="xT", bufs=3 * KD))
    h1pool = ctx.enter_context(tc.tile_pool(name="h1T", bufs=2 * KH))
    h2pool = ctx.enter_context(tc.tile_pool(name="h2T", bufs=2 * KH2))
    tpool = ctx.enter_context(tc.tile_pool(name="tmp", bufs=3))
    opool = ctx.enter_context(tc.tile_pool(name="ostage", bufs=2))
    mmpool = ctx.enter_context(tc.tile_pool(name="mm", bufs=3, space="PSUM"))
    mm2pool = ctx.enter_context(tc.tile_pool(name="mm2", bufs=KH2, space="PSUM"))
    mm3pool = ctx.enter_context(tc.tile_pool(name="mm3", bufs=1, space="PSUM"))
    xbd = ctx.enter_context(tc.tile_pool(name="xbd", bufs=2, space="DRAM"))

    # Weights+biases arrive byte-packed in TWO dram tensors loaded with ONE
    # DMA each: concurrent transfers on a queue dilute each other's
    # bandwidth, so the fewer/more-sequential the critical loads, the
    # earlier mm1/mm2 can start. pack1 = w1|b1c|b1s, pack2 = w2|b2c|w3|b3.
    U8 = mybir.dt.uint8
    pk1s = wpool.tile([128, PK1_BYTES], U8)
    pk2s = wpool.tile([128, PK2_BYTES], U8)
    w1s = pk1s[:, : KH * KD * 256].bitcast(BF16)   # [p, i*512+k*128+m] = W1[k*128+p, i*128+m]
    b1cs = pk1s[:, KH * KD * 256 : KH * KD * 256 + 4 * KH].bitcast(F32)
    b1ss = pk1s[:, KH * KD * 256 + 4 * KH :].bitcast(F32)
    w2s = pk2s[:, : KH2 * KH * 256].bitcast(BF16)  # [p, j*1024+k*128+m] = W2[k*128+p, j*128+m]
    o2 = KH2 * KH * 256
    b2cs = pk2s[:, o2 : o2 + 4 * KH2].bitcast(F32)
    w3s = pk2s[:, o2 + 4 * KH2 : o2 + 4 * KH2 + 2 * KH2 * C].bitcast(BF16)
    b3cs = pk2s[:, o2 + 4 * KH2 + 2 * KH2 * C :].bitcast(F32)[0:C]  # [10,1]
    warm = wpool.tile([128, 512], BF16)            # warm-up operand

    # gpsimd/SWDGE queue: cast0, memset, pack1, then remaining casts.
    # xbd bufs=2 head-of-line blocks the queue at cast(c+2) until chunk c's
    # transposes have read xb — at most one cast in flight (no bandwidth
    # dilution) while cast(c+1) overlaps chunk c's transposes.
    xbs = []
    row0 = 0
    for c, NB in enumerate(CHUNKS):
        rows = slice(row0, row0 + NB)
        row0 += NB
        xb = xbd.tile([NBMAX, D], BF16, tag="xb", name="xb")[:NB]
        nc.gpsimd.dma_start(out=xb[:], in_=x[rows, :])
        xbs.append((xb, rows))
        if c == 0:
            nc.gpsimd.memset(warm[:], 1.0)
            nc.gpsimd.dma_start(out=pk1s[:], in_=pk1[:])

    # scalar HWDGE queue: pack2 alone up front (full rate, done before mm2
    # needs it), stores appended per-chunk later.
    nc.scalar.dma_start(out=pk2s[:], in_=pk2[:])

    # Warm-up matmuls: garbage-in, never-read-out; just PE activity so the
    # HAM clock gate opens before the first real matmul.
    warmps = mm3pool.tile([128, NBMAX], F32, tag="ps3", name="warmps")
    for _ in range(WARM_MM):
        nc.tensor.matmul(warmps[:, :512], warm[:, :128], warm[:], start=True, stop=True)

    for c, NB in enumerate(CHUNKS):
        xb, rows = xbs[c]

        # per-k-slice xbar transposes into SBUF (sync HWDGE only)
        xT = []
        for k in range(KD):
            xt = xpool.tile([128, NBMAX], BF16, tag="xt", name="xt")[:, :NB]
            nc.sync.dma_start(out=xt[:], in_=xb[:, k * 128 : (k + 1) * 128], transpose=True)
            xT.append(xt)

        # mm2 accumulators for this chunk (4 PSUM banks, live whole chunk)
        ps2 = [
            mm2pool.tile([128, NBMAX], F32, tag="ps2", name="ps2")[:, :NB]
            for _ in range(KH2)
        ]
        h1T = []

        def mm1_tile(i):
            ps = mmpool.tile([128, NBMAX], F32, tag="ps", name="ps")[:, :NB]
            for k in range(KD):
                nc.tensor.matmul(
                    ps[:],
                    w1s[:, i * 512 + k * 128 : i * 512 + (k + 1) * 128],
                    xT[k][:],
                    start=(k == 0),
                    stop=(k == KD - 1),
                )
            t = tpool.tile([128, NBMAX], BF16, tag="t", name="t")[:, :NB]
            a = tpool.tile([128, NBMAX], BF16, tag="a", name="a")[:, :NB]
            nc.scalar.activation(t[:], ps[:], AF.Tanh, bias=b1cs[:, i : i + 1], scale=1.0)
            nc.scalar.activation(a[:], ps[:], AF.Relu, bias=b1ss[:, i : i + 1], scale=0.9)
            m = tpool.tile([128, NBMAX], BF16, tag="m", name="m")[:, :NB]
            u = tpool.tile([128, NBMAX], BF16, tag="u", name="u")[:, :NB]
            nc.vector.tensor_scalar(m[:], t[:], 0.0, 0.4, ALU.min, ALU.mult)
            nc.vector.scalar_tensor_tensor(u[:], t[:], 0.1, a[:], ALU.mult, ALU.add)
            h1 = h1pool.tile([128, NBMAX], BF16, tag="h1", name="h1")[:, :NB]
            nc.vector.tensor_add(h1[:], u[:], m[:])
            h1T.append(h1)

        def mm2_k(k):
            for j in range(KH2):
                nc.tensor.matmul(
                    ps2[j][:],
                    w2s[:, j * 1024 + k * 128 : j * 1024 + (k + 1) * 128],
                    h1T[k][:],
                    start=(k == 0),
                    stop=(k == KH - 1),
                )

        # interleaved schedule: mm2(k) rides LAG tiles behind mm1(i)
        for i in range(KH):
            mm1_tile(i)
            if i >= LAG:
                mm2_k(i - LAG)
        for k in range(KH - LAG, KH):
            mm2_k(k)

        h2T = []
        for j in range(KH2):
            h2 = h2pool.tile([128, NBMAX], BF16, tag="h2", name="h2")[:, :NB]
            nc.scalar.activation(h2[:], ps2[j][:], AF.Relu, bias=b2cs[:, j : j + 1], scale=1.0)
            h2T.append(h2)

        # ---- mm3: out^T [10, NB] = W3^T @ h2 + b3 ----
        ps3 = mm3pool.tile([128, NBMAX], F32, tag="ps3", name="ps3")
        if MM3_TILED:
            # Two k-pairs run concurrently in PE column groups 0 and 32.
            # The strips stay partition-aligned through SBUF and DRAM; the
            # host adds them during gather (DVE can't read two PSUM strips
            # at different partition bases in one op).
            stage = opool.tile([64, NBMAX], F32, tag="stage", name="stage")
            for k in range(2):
                nc.tensor.matmul(
                    ps3[0:C, :NB],
                    w3s[:, k * C : (k + 1) * C],
                    h2T[k][:],
                    start=(k == 0),
                    stop=(k == 1),
                    tile_position=(0, 0),
                )
            for k in range(2, 4):
                nc.tensor.matmul(
                    ps3[32 : 32 + C, :NB],
                    w3s[:, k * C : (k + 1) * C],
                    h2T[k][:],
                    start=(k == 2),
                    stop=(k == 3),
                    tile_position=(0, 32),
                )
            nc.vector.tensor_scalar_add(stage[0:C, :NB], ps3[0:C, :NB], b3cs[:])
            nc.vector.tensor_scalar_add(stage[32 : 32 + C, :NB], ps3[32 : 32 + C, :NB], 0.0)
            # stores on the scalar HWDGE queue (weights are done by now;
            # keeps gpsimd/sync queues pure prefetch)
            nc.scalar.dma_start(out=outT[0:C, rows], in_=stage[0:C, :NB])
            nc.scalar.dma_start(out=outT[C : 2 * C, rows], in_=stage[32 : 32 + C, :NB])
        else:
            stage = opool.tile([64, NBMAX], F32, tag="stage", name="stage")
            for k in range(KH2):
                nc.tensor.matmul(
                    ps3[0:C, :NB],
                    w3s[:, k * C : (k + 1) * C],
                    h2T[k][:],
                    start=(k == 0),
                    stop=(k == KH2 - 1),
                )
            nc.vector.tensor_scalar_add(stage[0:C, :NB], ps3[0:C, :NB], b3cs[:])
            nc.scalar.dma_start(out=outT[0:C, rows], in_=stage[0:C, :NB])


_CACHED = None


def _build():
    global _CACHED
    if _CACHED is not None:
        return _CACHED
    nc = bacc.Bacc(
        "TRN2",
        target_bir_lowering=False,
        debug=False,
        enable_asserts=False,
        num_devices=N_CORES,
    )
    x = nc.dram_tensor("x", [BL, D], F32, kind="ExternalInput").ap()
    pk1 = nc.dram_tensor("pk1", [128, PK1_BYTES], mybir.dt.uint8, kind="ExternalInput").ap()
    pk2 = nc.dram_tensor("pk2", [128, PK2_BYTES], mybir.dt.uint8, kind="ExternalInput").ap()
    outT = nc.dram_tensor("outT", [2 * C, BL], F32, kind="ExternalOutput").ap()

    from contextlib import ExitStack

    with tile.TileContext(nc) as tc, ExitStack() as ctx:
        _body(ctx, tc, [outT], [x, pk1, pk2])
    nc.compile()
    _CACHED = nc
    return nc


def _prep_weights(W1, b1, W2, b2, W3, b3):
    bf = ml_dtypes.bfloat16
    u8 = np.uint8
    # i-major layouts: each mm1/mm2 output tile's weights are contiguous
    w1h = np.ascontiguousarray(
        W1.astype(bf).reshape(KD, 128, KH, 128).transpose(1, 2, 0, 3).reshape(128, -1)
    )
    w2h = np.ascontiguousarray(
        W2.astype(bf).reshape(KH, 128, KH2, 128).transpose(1, 2, 0, 3).reshape(128, -1)
    )
    w3h = np.ascontiguousarray(
        W3.astype(bf).reshape(KH2, 128, C).transpose(1, 0, 2).reshape(128, KH2 * C)
    )
    b1f = b1.astype(np.float32)
    b1ch = np.ascontiguousarray(b1f.reshape(KH, 128).T)
    b1sh = np.ascontiguousarray((0.9 * b1f).reshape(KH, 128).T)
    b2ch = np.ascontiguousarray(b2.astype(np.float32).reshape(KH2, 128).T)
    b3pad = np.zeros((128, 1), np.float32)
    b3pad[:C, 0] = b3.astype(np.float32)
    pk1 = np.ascontiguousarray(
        np.hstack([w1h.view(u8), b1ch.view(u8), b1sh.view(u8)])
    )
    pk2 = np.ascontiguousarray(
        np.hstack([w2h.view(u8), b2ch.view(u8), w3h.view(u8), b3pad.view(u8)])
    )
    assert pk1.shape == (128, PK1_BYTES) and pk2.shape == (128, PK2_BYTES)
    return pk1, pk2


def _make_in_maps(x, W1, b1, W2, b2, W3, b3):
    x = np.asarray(x, dtype=np.float32)
    pk1, pk2 = _prep_weights(
        np.asarray(W1), np.asarray(b1), np.asarray(W2), np.asarray(b2),
        np.asarray(W3), np.asarray(b3),
    )
    return [
        {
            "x": np.ascontiguousarray(x[i * BL : (i + 1) * BL]),
            "pk1": pk1, "pk2": pk2,
        }
        for i in range(N_CORES)
    ]


def _gather(core_outs):
    # outT carries two partition-aligned partial strips (PE column groups);
    # sum them here. The untiled path leaves strip 1 at its donated zeros.
    return np.concatenate(
        [np.ascontiguousarray((o["outT"][:C] + o["outT"][C:]).T) for o in core_outs],
        axis=0,
    ).astype(np.float32, copy=False)


def kernel(x, W1, b1, W2, b2, W3, b3):
    nc = _build()
    in_maps = _make_in_maps(x, W1, b1, W2, b2, W3, b3)
    res = run_bass_kernel_spmd(nc, in_maps, core_ids=list(range(N_CORES))).results
    return _gather(res)


# revision 18
# speedup vs baseline: 1.0701x; 1.0701x over previous
"""Trainium2 Bass kernel for nn_ComplexNN (3-layer MLP, blended tanh act).

  h1 = blend_act(x @ W1 + b1);  blend_act(z) = z>0 ? 0.9z+0.1tanh(z) : 0.5tanh(z)
  h2 = relu(h1 @ W2 + b2)
  out = h2 @ W3 + b3

Data-parallel over 8 NeuronCores: each core takes 4096 rows of x, weights
replicated. Fully fused on-chip; matmuls in bf16 with fp32 PSUM accumulate.

Layout: activations are kept feature-on-partitions (h1^T, h2^T) so each
matmul's contraction dim lands on partitions with no intermediate
transposes. x is cast fp32->bf16 via SWDGE DMA (DRAM->DRAM, whole-chunk)
then DMA-xbar-transposed (DRAM->SBUF). out^T [10, 4096] goes to DRAM and
the host transposes during the unshard/gather step.

Queue plan (no head-of-line blocking): gpsimd/SWDGE carries ONLY the x
casts (pure prefetch FIFO), sync HWDGE carries ONLY the xbar transposes,
scalar HWDGE carries weights (front) then output stores.

Pipeline: uniform 512-col chunks (256 head/tail). mm2's k-accumulation is
interleaved into the mm1 i-loop (lag 3) so the tensor queue never drains
at chunk tails. mm3 packs its tiny M=10 output into two PE column groups
(k-pairs run concurrently at col offsets 0/32), reduced on DVE.
A few warm-up matmuls on a memset tile run during ingest to lift the HAM
clock gate (PE starts at 1.2 GHz, un-throttles after ~3.4us of activity).

blend_act decomposition (t = tanh(z)):
  blend(z) = 0.9*relu(z) + 0.1*t + 0.4*min(t, 0)
ACT: t = Tanh(psum + b1);  a = Relu(0.9*psum + 0.9*b1)
DVE: m = (t min 0)*0.4 ;  u = 0.1*t + a (STT);  h1 = u + m
"""

import sys

sys.path.insert(0, "/opt/trn_rl_repo")

import ml_dtypes
import numpy as np

import concourse.bass as bass
import concourse.mybir as mybir
import concourse.tile as tile
from concourse import bacc
from concourse.bass_utils import run_bass_kernel_spmd

N_CORES = 8
B, D, H, H2, C = 32768, 512, 1024, 512, 10
BL = B // N_CORES  # rows per core = 4096
CHUNKS = [512] * 8
assert sum(CHUNKS) == BL
KD = D // 128      # 4  k-tiles for mm1
KH = H // 128      # 8  k-tiles for mm2 / h-tiles of h1
KH2 = H2 // 128    # 4  k-tiles for mm3 / h2-tiles of h2
NBMAX = max(CHUNKS)
PK1_BYTES = KH * KD * 256 + 8 * KH          # w1 bf16 | b1c f32 | b1s f32
PK2_BYTES = KH2 * KH * 256 + 4 * KH2 + 2 * KH2 * C + 4  # w2 | b2c | w3 | b3
WARM_MM = 6        # warm-up matmuls to release the HAM clock gate
LAG = 3            # mm2(k) issues after mm1(k+LAG)
MM3_TILED = True   # mm3 via 2 PE column groups

F32 = mybir.dt.float32
BF16 = mybir.dt.bfloat16
AF = mybir.ActivationFunctionType
ALU = mybir.AluOpType


def _body(ctx, tc, outs, ins):
    nc = tc.nc
    x, pk1, pk2 = ins
    (outT,) = outs

    wpool = ctx.enter_context(tc.tile_pool(name="weights", bufs=1))
    xpool = ctx.enter_context(tc.tile_pool(name="xT", bufs=2 * KD))
    h1pool = ctx.enter_context(tc.tile_pool(name="h1T", bufs=2 * KH))
    h2pool = ctx.enter_context(tc.tile_pool(name="h2T", bufs=2 * KH2))
    tpool = ctx.enter_context(tc.tile_pool(name="tmp", bufs=3))
    opool = ctx.enter_context(tc.tile_pool(name="ostage", bufs=2))
    mmpool = ctx.enter_context(tc.tile_pool(name="mm", bufs=3, space="PSUM"))
    mm2pool = ctx.enter_context(tc.tile_pool(name="mm2", bufs=KH2, space="PSUM"))
    mm3pool = ctx.enter_context(tc.tile_pool(name="mm3", bufs=1, space="PSUM"))
    xbd = ctx.enter_context(tc.tile_pool(name="xbd", bufs=2, space="DRAM"))

    # Weights+biases arrive byte-packed in TWO dram tensors loaded with ONE
    # DMA each: concurrent transfers on a queue dilute each other's
    # bandwidth, so the fewer/more-sequential the critical loads, the
    # earlier mm1/mm2 can start. pack1 = w1|b1c|b1s, pack2 = w2|b2c|w3|b3.
    U8 = mybir.dt.uint8
    pk1s = wpool.tile([128, PK1_BYTES], U8)
    pk2s = wpool.tile([128, PK2_BYTES], U8)
    w1s = pk1s[:, : KH * KD * 256].bitcast(BF16)   # [p, i*512+k*128+m] = W1[k*128+p, i*128+m]
    b1cs = pk1s[:, KH * KD * 256 : KH * KD * 256 + 4 * KH].bitcast(F32)
    b1ss = pk1s[:, KH * KD * 256 + 4 * KH :].bitcast(F32)
    w2s = pk2s[:, : KH2 * KH * 256].bitcast(BF16)  # [p, j*1024+k*128+m] = W2[k*128+p, j*128+m]
    o2 = KH2 * KH * 256
    b2cs = pk2s[:, o2 : o2 + 4 * KH2].bitcast(F32)
    w3s = pk2s[:, o2 + 4 * KH2 : o2 + 4 * KH2 + 2 * KH2 * C].bitcast(BF16)
    b3cs = pk2s[:, o2 + 4 * KH2 + 2 * KH2 * C :].bitcast(F32)[0:C]  # [10,1]
    warm = wpool.tile([128, 512], BF16)            # warm-up operand

    # gpsimd/SWDGE queue: cast0, memset, pack1, then remaining casts.
    # xbd bufs=2 head-of-line blocks the queue at cast(c+2) until chunk c's
    # transposes have read xb — at most one cast in flight (no bandwidth
    # dilution) while cast(c+1) overlaps chunk c's transposes.
    xbs = []
    row0 = 0
    for c, NB in enumerate(CHUNKS):
        rows = slice(row0, row0 + NB)
        row0 += NB
        xb = xbd.tile([NBMAX, D], BF16, tag="xb", name="xb")[:NB]
        nc.gpsimd.dma_start(out=xb[:], in_=x[rows, :])
        xbs.append((xb, rows))
        if c == 0:
            nc.gpsimd.memset(warm[:], 1.0)
            nc.gpsimd.dma_start(out=pk1s[:], in_=pk1[:])

    # scalar HWDGE queue: pack2 alone up front (full rate, done before mm2
    # needs it), stores appended per-chunk later.
    nc.scalar.dma_start(out=pk2s[:], in_=pk2[:])

    # Warm-up matmuls: garbage-in, never-read-out; just PE activity so the
    # HAM clock gate opens before the first real matmul.
    warmps = mm3pool.tile([128, NBMAX], F32, tag="ps3", name="warmps")
    for _ in range(WARM_MM):
        nc.tensor.matmul(warmps[:, :512], warm[:, :128], warm[:], start=True, stop=True)

    for c, NB in enumerate(CHUNKS):
        xb, rows = xbs[c]

        # per-k-slice xbar transposes into SBUF, split across both HWDGE
        # queues so the ~1.2us per-transpose dispatch doesn't serialize
        xT = []
        for k in range(KD):
            xt = xpool.tile([128, NBMAX], BF16, tag="xt", name="xt")[:, :NB]
            eng = nc.sync if k < 2 else nc.scalar
            eng.dma_start(out=xt[:], in_=xb[:, k * 128 : (k + 1) * 128], transpose=True)
            xT.append(xt)

        # mm2 accumulators for this chunk (4 PSUM banks, live whole chunk)
        ps2 = [
            mm2pool.tile([128, NBMAX], F32, tag="ps2", name="ps2")[:, :NB]
            for _ in range(KH2)
        ]
        h1T = []

        def mm1_tile(i):
            ps = mmpool.tile([128, NBMAX], F32, tag="ps", name="ps")[:, :NB]
            for k in range(KD):
                nc.tensor.matmul(
                    ps[:],
                    w1s[:, i * 512 + k * 128 : i * 512 + (k + 1) * 128],
                    xT[k][:],
                    start=(k == 0),
                    stop=(k == KD - 1),
                )
            t = tpool.tile([128, NBMAX], BF16, tag="t", name="t")[:, :NB]
            a = tpool.tile([128, NBMAX], BF16, tag="a", name="a")[:, :NB]
            nc.scalar.activation(t[:], ps[:], AF.Tanh, bias=b1cs[:, i : i + 1], scale=1.0)
            nc.scalar.activation(a[:], ps[:], AF.Relu, bias=b1ss[:, i : i + 1], scale=0.9)
            m = tpool.tile([128, NBMAX], BF16, tag="m", name="m")[:, :NB]
            u = tpool.tile([128, NBMAX], BF16, tag="u", name="u")[:, :NB]
            nc.vector.tensor_scalar(m[:], t[:], 0.0, 0.4, ALU.min, ALU.mult)
            nc.vector.scalar_tensor_tensor(u[:], t[:], 0.1, a[:], ALU.mult, ALU.add)
            h1 = h1pool.tile([128, NBMAX], BF16, tag="h1", name="h1")[:, :NB]
            nc.vector.tensor_add(h1[:], u[:], m[:])
            h1T.append(h1)

        def mm2_k(k):
            for j in range(KH2):
                nc.tensor.matmul(
                    ps2[j][:],
                    w2s[:, j * 1024 + k * 128 : j * 1024 + (k + 1) * 128],
                    h1T[k][:],
                    start=(k == 0),
                    stop=(k == KH - 1),
                )

        # interleaved schedule: mm2(k) rides LAG tiles behind mm1(i)
        for i in range(KH):
            mm1_tile(i)
            if i >= LAG:
                mm2_k(i - LAG)
        for k in range(KH - LAG, KH):
            mm2_k(k)

        h2T = []
        for j in range(KH2):
            h2 = h2pool.tile([128, NBMAX], BF16, tag="h2", name="h2")[:, :NB]
            nc.scalar.activation(h2[:], ps2[j][:], AF.Relu, bias=b2cs[:, j : j + 1], scale=1.0)
            h2T.append(h2)

        # ---- mm3: out^T [10, NB] = W3^T @ h2 + b3 ----
        ps3 = mm3pool.tile([128, NBMAX], F32, tag="ps3", name="ps3")
        if MM3_TILED:
            # Two k-pairs run concurrently in PE column groups 0 and 32.
            # The strips stay partition-aligned through SBUF and DRAM; the
            # host adds them during gather (DVE can't read two PSUM strips
            # at different partition bases in one op).
            stage = opool.tile([64, NBMAX], F32, tag="stage", name="stage")
            for k in range(2):
                nc.tensor.matmul(
                    ps3[0:C, :NB],
                    w3s[:, k * C : (k + 1) * C],
                    h2T[k][:],
                    start=(k == 0),
                    stop=(k == 1),
                    tile_position=(0, 0),
                )
            for k in range(2, 4):
                nc.tensor.matmul(
                    ps3[32 : 32 + C, :NB],
                    w3s[:, k * C : (k + 1) * C],
                    h2T[k][:],
                    start=(k == 2),
                    stop=(k == 3),
                    tile_position=(0, 32),
                )
            nc.vector.tensor_scalar_add(stage[0:C, :NB], ps3[0:C, :NB], b3cs[:])
            nc.vector.tensor_scalar_add(stage[32 : 32 + C, :NB], ps3[32 : 32 + C, :NB], 0.0)
            # single merged store (partitions 0..41; host reads the two
            # strips) — fewer DMAs keeps the shared semaphore pool calm
            nc.scalar.dma_start(out=outT[:, rows], in_=stage[0 : 32 + C, :NB])
        else:
            stage = opool.tile([64, NBMAX], F32, tag="stage", name="stage")
            for k in range(KH2):
                nc.tensor.matmul(
                    ps3[0:C, :NB],
                    w3s[:, k * C : (k + 1) * C],
                    h2T[k][:],
                    start=(k == 0),
                    stop=(k == KH2 - 1),
                )
            nc.vector.tensor_scalar_add(stage[0:C, :NB], ps3[0:C, :NB], b3cs[:])
            nc.scalar.dma_start(out=outT[0:C, rows], in_=stage[0:C, :NB])


_CACHED = None


def _build():
    global _CACHED
    if _CACHED is not None:
        return _CACHED
    nc = bacc.Bacc(
        "TRN2",
        target_bir_lowering=False,
        debug=False,
        enable_asserts=False,
        num_devices=N_CORES,
    )
    x = nc.dram_tensor("x", [BL, D], F32, kind="ExternalInput").ap()
    pk1 = nc.dram_tensor("pk1", [128, PK1_BYTES], mybir.dt.uint8, kind="ExternalInput").ap()
    pk2 = nc.dram_tensor("pk2", [128, PK2_BYTES], mybir.dt.uint8, kind="ExternalInput").ap()
    outT = nc.dram_tensor("outT", [32 + C, BL], F32, kind="ExternalOutput").ap()

    from contextlib import ExitStack

    with tile.TileContext(nc) as tc, ExitStack() as ctx:
        _body(ctx, tc, [outT], [x, pk1, pk2])
    nc.compile()
    _CACHED = nc
    return nc


def _prep_weights(W1, b1, W2, b2, W3, b3):
    bf = ml_dtypes.bfloat16
    u8 = np.uint8
    # i-major layouts: each mm1/mm2 output tile's weights are contiguous
    w1h = np.ascontiguousarray(
        W1.astype(bf).reshape(KD, 128, KH, 128).transpose(1, 2, 0, 3).reshape(128, -1)
    )
    w2h = np.ascontiguousarray(
        W2.astype(bf).reshape(KH, 128, KH2, 128).transpose(1, 2, 0, 3).reshape(128, -1)
    )
    w3h = np.ascontiguousarray(
        W3.astype(bf).reshape(KH2, 128, C).transpose(1, 0, 2).reshape(128, KH2 * C)
    )
    b1f = b1.astype(np.float32)
    b1ch = np.ascontiguousarray(b1f.reshape(KH, 128).T)
    b1sh = np.ascontiguousarray((0.9 * b1f).reshape(KH, 128).T)
    b2ch = np.ascontiguousarray(b2.astype(np.float32).reshape(KH2, 128).T)
    b3pad = np.zeros((128, 1), np.float32)
    b3pad[:C, 0] = b3.astype(np.float32)
    pk1 = np.ascontiguousarray(
        np.hstack([w1h.view(u8), b1ch.view(u8), b1sh.view(u8)])
    )
    pk2 = np.ascontiguousarray(
        np.hstack([w2h.view(u8), b2ch.view(u8), w3h.view(u8), b3pad.view(u8)])
    )
    assert pk1.shape == (128, PK1_BYTES) and pk2.shape == (128, PK2_BYTES)
    return pk1, pk2


def _make_in_maps(x, W1, b1, W2, b2, W3, b3):
    x = np.asarray(x, dtype=np.float32)
    pk1, pk2 = _prep_weights(
        np.asarray(W1), np.asarray(b1), np.asarray(W2), np.asarray(b2),
        np.asarray(W3), np.asarray(b3),
    )
    return [
        {
            "x": np.ascontiguousarray(x[i * BL : (i + 1) * BL]),
            "pk1": pk1, "pk2": pk2,
        }
        for i in range(N_CORES)
    ]


def _gather(core_outs):
    # outT carries two partition-aligned partial strips (PE column groups)
    # at rows 0:C and 32:32+C; sum them here. The untiled path leaves the
    # second strip at its donated zeros.
    return np.concatenate(
        [
            np.ascontiguousarray((o["outT"][:C] + o["outT"][32 : 32 + C]).T)
            for o in core_outs
        ],
        axis=0,
    ).astype(np.float32, copy=False)


def kernel(x, W1, b1, W2, b2, W3, b3):
    nc = _build()
    in_maps = _make_in_maps(x, W1, b1, W2, b2, W3, b3)
    res = run_bass_kernel_spmd(nc, in_maps, core_ids=list(range(N_CORES))).results
    return _gather(res)
